# revision 1
# baseline (speedup 1.0000x reference)
"""Trainium2 Bass kernel for nn_BG_ALRT_62921270886438 (moe_routing).

Sharding: cores 0-3 replicate batch 0, cores 4-7 replicate batch 1 (the step
loop runs per-batch on every core with zero collectives); the lm_head matmul
is vocab-sharded 4 ways within each batch group. Exploits w_eff sparsity:
only layers with exp(-|depth - t|) > 0.15 are computed each step.

Self-contained: only numpy + the concourse toolchain on sys.path.
"""
import os

import numpy as np

import concourse.bacc as bacc
import concourse.tile as tile
from concourse import mybir
from concourse.alu_op_type import AluOpType
from concourse.bass_utils import run_bass_kernel_spmd

AF = mybir.ActivationFunctionType
F32 = mybir.dt.float32
F32R = mybir.dt.float32r

B, T, E, G, GD, L, N, V = 2, 256, 512, 8, 64, 8, 64, 50257
HD = GD // 2          # 32, rope half
NC = 8                # cores
VSH = 4               # vocab shards per batch group
VW = (V + VSH - 1) // VSH          # 12565 raw shard width
VQ = ((VW + 511) // 512) * 512     # 12800 padded shard width
EPS = float(np.finfo(np.float32).eps)
KT = E // 128         # 4 contraction tiles over E
PAIRS = 4             # node pairs per layer (8 nodes)

_PROGRAM_CACHE = {}


def _trunc(a):
    """Truncate fp32 mantissa to fp32r (low 12 bits zeroed), matching HW."""
    a = np.ascontiguousarray(a, dtype=np.float32)
    b = np.frombuffer(a.tobytes(), dtype=np.uint32) & np.uint32(0xFFFFF000)
    return np.frombuffer(b.tobytes(), dtype=np.float32).reshape(a.shape).copy()


def _build_program(active_sets):
    """active_sets: tuple of tuples — active layer list per step."""
    nc = bacc.Bacc("TRN2", target_bir_lowering=False, debug=False, num_devices=NC)
    n_ls = max(sum(len(a) for a in active_sets), 1)

    d_x0t = nc.dram_tensor("x0t", [E, T], F32, kind="ExternalInput")
    d_adw = nc.dram_tensor("adw", [L, 128, KT * 512], F32, kind="ExternalInput")
    d_qkw = nc.dram_tensor("qkw", [L, 128, 512], F32, kind="ExternalInput")
    d_vw = nc.dram_tensor("vw", [L, 128, 512], F32, kind="ExternalInput")
    d_fcw = nc.dram_tensor("fcw", [L, 128, 1024], F32, kind="ExternalInput")
    d_cr = nc.dram_tensor("cstr", [128, 640], F32, kind="ExternalInput")
    d_cf = nc.dram_tensor("cstf", [128, 648], F32, kind="ExternalInput")
    d_wap = nc.dram_tensor("wapP", [128, L * PAIRS], F32, kind="ExternalInput")
    d_waw = nc.dram_tensor("wawP", [128, n_ls * PAIRS], F32, kind="ExternalInput")
    d_wmw = nc.dram_tensor("wmwP", [128, n_ls * PAIRS], F32, kind="ExternalInput")
    d_rw = nc.dram_tensor("rwP", [128, KT], F32, kind="ExternalInput")
    d_rb = nc.dram_tensor("rbias", [1, 1], F32, kind="ExternalInput")
    d_lm = nc.dram_tensor("lmt", [E, VQ], F32, kind="ExternalInput")
    d_out = nc.dram_tensor("out_lg", [T, VQ], F32, kind="ExternalOutput")

    NVT = VQ // 512   # 25 vocab tiles of 512
    NTT = T // 128    # 2 token tiles

    with tile.TileContext(nc) as tc:
        with tc.tile_pool(name="cst", bufs=1) as cst, \
             tc.tile_pool(name="st", bufs=1) as st, \
             tc.tile_pool(name="wk", bufs=2) as wk, \
             tc.tile_pool(name="wk4", bufs=4) as wk4, \
             tc.tile_pool(name="adp", bufs=2) as adp, \
             tc.tile_pool(name="lmp", bufs=2) as lmp, \
             tc.tile_pool(name="ps6", bufs=6, space="PSUM") as ps6, \
             tc.tile_pool(name="ps1", bufs=1, space="PSUM") as ps1:

            # ---------------- constants / weights ----------------
            c_r = cst.tile([128, 640], F32R, tag="c_r", name="c_r")
            nc.sync.dma_start(c_r[:], d_cr.ap().bitcast(F32R))
            perm = c_r[:, 0:128]            # rope swap permutation
            oblk = c_r[:, 128:256]          # 1/64 block-diagonal(64) lhsT
            ocol = c_r[:, 256:320]          # (128,64) all ones
            orow128 = c_r[0:1, 256:384]     # (1,128) ones
            orow64 = c_r[0:1, 256:320]      # (1,64) ones
            oc1 = c_r[:, 384:385]           # (128,1) ones
            sel2 = c_r[0:2, 385:513]        # row0 -> rows 0:64, row1 -> rows 64:128
            oblk2 = c_r[:, 513:515]         # col0: 1/64 on rows 0:64; col1: rows 64:128

            c_f = cst.tile([128, 648], F32, tag="c_f", name="c_f")
            nc.sync.dma_start(c_f[:], d_cf.ap())
            C128 = c_f[:, 0:256]
            S128 = c_f[:, 256:512]
            tri = c_f[:, 512:640]
            one_f = c_f[0:1, 0:1]           # cos(0)=1.0, identity for transpose
            eps128 = c_f[:, 640:641]
            eps1 = c_f[0:1, 640:641]

            wap = cst.tile([128, L * PAIRS], F32, tag="wap", name="wap")
            nc.sync.dma_start(wap[:], d_wap.ap())
            waw = cst.tile([128, n_ls * PAIRS], F32, tag="waw", name="waw")
            nc.sync.dma_start(waw[:], d_waw.ap())
            wmw = cst.tile([128, n_ls * PAIRS], F32, tag="wmw", name="wmw")
            nc.sync.dma_start(wmw[:], d_wmw.ap())
            rw = cst.tile([128, KT], F32R, tag="rw", name="rw")
            nc.sync.dma_start(rw[:], d_rw.ap().bitcast(F32R))
            rbias = cst.tile([1, 1], F32, tag="rbias", name="rbias")
            nc.sync.dma_start(rbias[:], d_rb.ap())

            qkw, vw, fcw = [], [], []
            for l in range(L):
                q_t = cst.tile([128, 512], F32R, tag=f"qkw{l}", name=f"qkw{l}")
                nc.sync.dma_start(q_t[:], d_qkw.ap().bitcast(F32R)[l])
                qkw.append(q_t)
                v_t = cst.tile([128, 512], F32R, tag=f"vw{l}", name=f"vw{l}")
                nc.sync.dma_start(v_t[:], d_vw.ap().bitcast(F32R)[l])
                vw.append(v_t)
                f_t = cst.tile([128, 1024], F32R, tag=f"fcw{l}", name=f"fcw{l}")
                nc.sync.dma_start(f_t[:], d_fcw.ap().bitcast(F32R)[l])
                fcw.append(f_t)

            # ---------------- state ----------------
            xT = [st.tile([128, T], F32, tag=f"xT{k}", name=f"xT{k}") for k in range(KT)]
            xr = [st.tile([128, T], F32R, tag=f"xr{k}", name=f"xr{k}") for k in range(KT)]
            acc = [st.tile([128, T], F32, tag=f"acc{k}", name=f"acc{k}") for k in range(KT)]
            pcont = st.tile([1, T], F32, tag="pcont", name="pcont")
            pcr = st.tile([1, T], F32R, tag="pcr", name="pcr")
            nc.vector.memset(pcont[:], 1.0)
            nc.vector.memset(pcr[:].bitcast(F32), 1.0)
            for k in range(KT):
                nc.gpsimd.memset(acc[k][:], 0.0)

            # ---------------- initial x = rms(wte[idx]) ----------------
            x0 = []
            for k in range(KT):
                x0k = wk.tile([128, T], F32, tag=f"x0_{k}", name=f"x0_{k}")
                nc.sync.dma_start(x0k[:], d_x0t.ap()[k * 128:(k + 1) * 128, :])
                x0.append(x0k)
            p_ms = ps6.tile([1, T], F32, tag="ps", name="ps")
            for k in range(KT):
                sq = wk.tile([128, T], F32R, tag="sq0", name="sq0")
                nc.scalar.activation(sq[:], x0[k][:], AF.Square)
                nc.tensor.matmul(p_ms[:], oc1, sq[:], start=(k == 0), stop=(k == KT - 1))
            rrow = wk.tile([1, T], F32, tag="rrow", name="rrow")
            nc.scalar.activation(rrow[:], p_ms[:], AF.Sqrt, bias=eps1, scale=1.0 / E)
            rrec = wk.tile([1, T], F32R, tag="rrec", name="rrec")
            with nc.allow_low_precision(reason="fp32r broadcast operand"):
                nc.vector.reciprocal(rrec[:], rrow[:])   # rsqrt(mean+eps)
            p_rb0 = ps6.tile([128, T], F32, tag="ps", name="ps")
            nc.tensor.matmul(p_rb0[:], orow128, rrec[:], start=True, stop=True)
            for k in range(KT):
                nc.vector.tensor_tensor(xT[k][:], x0[k][:], p_rb0[:], AluOpType.mult)
                nc.vector.tensor_copy(xr[k][:], xT[k][:])

            # ---------------- step loop ----------------
            ls_idx = 0
            for t, layers in enumerate(active_sets):
                for l in layers:
                    adl = adp.tile([128, KT * 512], F32R, tag="adl", name="adl")
                    nc.sync.dma_start(adl[:], d_adw.ap().bitcast(F32R)[l])
                    for p in range(PAIRS):
                        rows_e, rows_o = slice(0, 64), slice(64, 128)
                        node_rc = ((rows_e, (0, 0)), (rows_o, (64, 0)))

                        # xi = adapters @ x (+ x)
                        p_xi = ps6.tile([128, T], F32, tag="ps", name="ps")
                        for k in range(KT):
                            nc.tensor.matmul(
                                p_xi[:], adl[:, k * 512 + p * 128: k * 512 + (p + 1) * 128],
                                xr[k][:], start=(k == 0), stop=(k == KT - 1))
                        xi = wk.tile([128, T], F32R, tag="xi", name="xi")
                        nc.vector.tensor_tensor(xi[:], p_xi[:], xT[p][:], AluOpType.add)

                        # qk per node -> [q;k] psum
                        p_qk = []
                        for rows, tp in node_rc:
                            pq = ps6.tile([128, T], F32, tag="ps", name="ps")
                            nc.tensor.matmul(pq[:], qkw[l][rows, p * 128:(p + 1) * 128],
                                             xi[rows, :], start=True, stop=True,
                                             tile_position=tp)
                            p_qk.append(pq)

                        # rope + rms -> qt/kt pair tiles
                        qt = wk.tile([128, T], F32R, tag="qt", name="qt")
                        kt = wk.tile([128, T], F32R, tag="kt", name="kt")
                        for o in range(2):
                            qs = wk.tile([128, T], F32R, tag="qs", name="qs")
                            nc.scalar.copy(qs[:], p_qk[o][:])
                            p_sw = ps6.tile([128, T], F32, tag="ps", name="ps")
                            nc.tensor.matmul(p_sw[:], perm, qs[:], start=True, stop=True)
                            t1 = wk.tile([128, T], F32, tag="t1", name="t1")
                            nc.gpsimd.tensor_tensor(t1[:], qs[:].bitcast(F32), C128,
                                                    AluOpType.mult)
                            rop = wk.tile([128, T], F32, tag="rop", name="rop")
                            t2 = wk.tile([128, T], F32, tag="t2", name="t2")
                            nc.vector.tensor_tensor(t2[:], p_sw[:], S128, AluOpType.mult)
                            nc.vector.tensor_tensor(rop[:], t1[:], t2[:], AluOpType.add)
                            sqr = wk.tile([128, T], F32R, tag="sqr", name="sqr")
                            nc.scalar.activation(sqr[:], rop[:], AF.Square)
                            p_m = ps6.tile([128, T], F32, tag="ps", name="ps")
                            nc.tensor.matmul(p_m[:], oblk, sqr[:], start=True, stop=True)
                            srt = wk.tile([128, T], F32, tag="srt", name="srt")
                            nc.scalar.activation(srt[:], p_m[:], AF.Sqrt, bias=eps128)
                            rsq = wk.tile([128, T], F32, tag="rsq", name="rsq")
                            nc.vector.reciprocal(rsq[:], srt[:])
                            orows = rows_e if o == 0 else rows_o
                            nc.vector.tensor_tensor(qt[orows, :], rop[0:64, :],
                                                    rsq[0:64, :], AluOpType.mult)
                            nc.vector.tensor_tensor(kt[orows, :], rop[64:128, :],
                                                    rsq[64:128, :], AluOpType.mult)

                        # scores -> exp/mask -> em tiles
                        em0, em1 = [None, None], [None, None]
                        for o, (rows, tp) in enumerate(node_rc):
                            p_s0 = ps6.tile([128, T], F32, tag="ps", name="ps")
                            nc.tensor.matmul(p_s0[:], kt[rows, 0:128], qt[rows, :],
                                             start=True, stop=True, tile_position=tp)
                            p_s1 = ps6.tile([128, 128], F32, tag="ps", name="ps")
                            nc.tensor.matmul(p_s1[:], kt[rows, 128:256], qt[rows, 128:256],
                                             start=True, stop=True, tile_position=tp)
                            e0 = wk4.tile([128, T], F32R, tag="em0", name="em0")
                            tmp = wk.tile([128, 128], F32, tag="etmp", name="etmp")
                            nc.scalar.activation(tmp[:], p_s0[:, 0:128], AF.Exp, scale=0.125)
                            nc.gpsimd.tensor_tensor(e0[:, 0:128], tmp[:], tri, AluOpType.mult)
                            nc.scalar.activation(e0[:, 128:256], p_s0[:, 128:256],
                                                 AF.Exp, scale=0.125)
                            e1 = wk4.tile([128, 128], F32R, tag="em1", name="em1")
                            tmp2 = wk.tile([128, 128], F32, tag="etmp2", name="etmp2")
                            nc.scalar.activation(tmp2[:], p_s1[:], AF.Exp, scale=0.125)
                            nc.gpsimd.tensor_tensor(e1[:], tmp2[:], tri, AluOpType.mult)
                            em0[o], em1[o] = e0, e1

                        # v per node per s-tile
                        v_sb = [[None, None], [None, None]]
                        for o, (rows, tp) in enumerate(node_rc):
                            for s in range(2):
                                p_v = ps6.tile([128, 64], F32, tag="ps", name="ps")
                                nc.tensor.matmul(
                                    p_v[:], xi[rows, s * 128:(s + 1) * 128],
                                    vw[l][rows, p * 128 + o * 64: p * 128 + (o + 1) * 64],
                                    start=True, stop=True, tile_position=tp)
                                vt = wk4.tile([128, 64], F32R, tag="vt", name="vt")
                                nc.scalar.copy(vt[:], p_v[:])
                                v_sb[o][s] = vt

                        # att + colsum
                        p_atts = []
                        p_cs0 = ps1.tile([1, T], F32, tag="pcs0", name="pcs0")
                        p_cs1 = ps1.tile([1, T], F32, tag="pcs1", name="pcs1")
                        for o in range(2):
                            p_att = ps6.tile([64, T], F32, tag="ps", name="ps")
                            p_atts.append(p_att)
                            p_cs = p_cs0 if o == 0 else p_cs1
                            nc.tensor.matmul(p_att[:, 0:128], v_sb[o][0][:],
                                             em0[o][:, 0:128], start=True, stop=True)
                            nc.tensor.matmul(p_att[:, 128:256], v_sb[o][0][:],
                                             em0[o][:, 128:256], start=True, stop=False)
                            nc.tensor.matmul(p_att[:, 128:256], v_sb[o][1][:],
                                             em1[o][:], start=False, stop=True)
                            nc.tensor.matmul(p_cs[0:1, 0:128], oc1, em0[o][:, 0:128],
                                             start=True, stop=True)
                            nc.tensor.matmul(p_cs[0:1, 128:256], oc1, em0[o][:, 128:256],
                                             start=True, stop=False)
                            nc.tensor.matmul(p_cs[0:1, 128:256], oc1, em1[o][:],
                                             start=False, stop=True)

                        rc0 = wk.tile([1, T], F32R, tag="rc0", name="rc0")
                        rc1 = wk.tile([1, T], F32R, tag="rc1", name="rc1")
                        with nc.allow_low_precision(reason="fp32r broadcast operand"):
                            nc.vector.reciprocal(rc0[:], p_cs0[0:1, :])
                            nc.vector.reciprocal(rc1[:], p_cs1[0:1, :])
                        p_rbe = ps6.tile([64, T], F32, tag="ps", name="ps")
                        nc.tensor.matmul(p_rbe[:], orow64, rc0[:], start=True, stop=True)
                        p_rbo = ps6.tile([64, T], F32, tag="ps", name="ps")
                        nc.tensor.matmul(p_rbo[:], orow64, rc1[:], start=True, stop=True)
                        att_sb = wk.tile([128, T], F32, tag="att", name="att")
                        nc.scalar.copy(att_sb[0:64, :], p_atts[0][:])
                        nc.scalar.copy(att_sb[64:128, :], p_atts[1][:])
                        tt = wk.tile([128, T], F32, tag="tt", name="tt")
                        nc.vector.tensor_tensor(tt[0:64, :], att_sb[0:64, :], p_rbe[:],
                                                AluOpType.mult)
                        nc.vector.tensor_tensor(tt[64:128, :], att_sb[64:128, :], p_rbo[:],
                                                AluOpType.mult)

                        xim = wk.tile([128, T], F32R, tag="xim", name="xim")
                        nc.vector.scalar_tensor_tensor(
                            xim[:], tt[:], wap[:, l * PAIRS + p: l * PAIRS + p + 1],
                            xi[:], AluOpType.mult, AluOpType.add)
                        nc.vector.scalar_tensor_tensor(
                            acc[p][:], tt[:],
                            waw[:, ls_idx * PAIRS + p: ls_idx * PAIRS + p + 1],
                            acc[p][:], AluOpType.mult, AluOpType.add)

                        # mlp
                        p_srs = []
                        for o, (rows, tp) in enumerate(node_rc):
                            p_sr = ps6.tile([64, T], F32, tag="ps", name="ps")
                            p_srs.append(p_sr)
                            for h in range(2):
                                p_fc = ps6.tile([128, T], F32, tag="ps", name="ps")
                                nc.tensor.matmul(
                                    p_fc[:],
                                    fcw[l][rows, p * 256 + h * 128: p * 256 + (h + 1) * 128],
                                    xim[rows, :], start=True, stop=True, tile_position=tp)
                                frel = wk.tile([128, T], F32R, tag="frel", name="frel")
                                nc.scalar.activation(frel[:], p_fc[:], AF.Relu)
                                rsq2 = wk.tile([128, T], F32R, tag="rsq2", name="rsq2")
                                nc.scalar.activation(rsq2[:], frel[:], AF.Square)
                                nc.tensor.matmul(p_sr[:], ocol, rsq2[:],
                                                 start=(h == 0), stop=(h == 1))
                        sqm = wk.tile([128, T], F32R, tag="sqm", name="sqm")
                        nc.scalar.activation(sqm[:], xim[:], AF.Square)
                        p_mq = ps6.tile([128, T], F32, tag="ps", name="ps")
                        nc.tensor.matmul(p_mq[:], oblk, sqm[:], start=True, stop=True)
                        pre = wk.tile([128, T], F32, tag="pre", name="pre")
                        nc.vector.tensor_scalar(pre[:], p_mq[:], 1.0, EPS,
                                                AluOpType.mult, AluOpType.add)
                        rec2 = wk.tile([128, T], F32, tag="rec2", name="rec2")
                        nc.vector.reciprocal(rec2[:], pre[:])
                        hm = wk.tile([128, T], F32, tag="hm", name="hm")
                        nc.vector.tensor_tensor(hm[0:64, :], p_srs[0][:], rec2[0:64, :],
                                                AluOpType.mult)
                        nc.vector.tensor_tensor(hm[64:128, :], p_srs[1][:], rec2[64:128, :],
                                                AluOpType.mult)
                        nc.vector.scalar_tensor_tensor(
                            acc[p][:], hm[:],
                            wmw[:, ls_idx * PAIRS + p: ls_idx * PAIRS + p + 1],
                            acc[p][:], AluOpType.mult, AluOpType.add)
                    ls_idx += 1

                # ---- x update + router ----
                p_pc = ps6.tile([128, T], F32, tag="ps", name="ps")
                nc.tensor.matmul(p_pc[:], orow128, pcr[:], start=True, stop=True)
                for k in range(KT):
                    upd = wk.tile([128, T], F32, tag="upd", name="upd")
                    nc.vector.tensor_tensor(upd[:], acc[k][:], p_pc[:], AluOpType.mult)
                    nc.vector.tensor_tensor(xT[k][:], upd[:], xT[k][:], AluOpType.add)
                    nc.vector.tensor_copy(xr[k][:], xT[k][:])
                    nc.gpsimd.memset(acc[k][:], 0.0)
                p_ph = ps6.tile([1, T], F32, tag="ps", name="ps")
                for k in range(KT):
                    nc.tensor.matmul(p_ph[:], rw[:, k:k + 1], xr[k][:],
                                     start=(k == 0), stop=(k == KT - 1))
                ph = wk.tile([1, T], F32, tag="ph", name="ph")
                nc.scalar.activation(ph[:], p_ph[:], AF.Sigmoid, bias=rbias[:])
                omp = wk.tile([1, T], F32, tag="omp", name="omp")
                nc.vector.tensor_scalar(omp[:], ph[:], -1.0, 1.0,
                                        AluOpType.mult, AluOpType.add)
                nc.vector.tensor_tensor(pcont[:], pcont[:], omp[:], AluOpType.mult)
                nc.vector.tensor_copy(pcr[:], pcont[:])

            # ---------------- final rms + lm_head ----------------
            p_mr = ps6.tile([1, T], F32, tag="ps", name="ps")
            for k in range(KT):
                sqf = wk.tile([128, T], F32R, tag="sqf", name="sqf")
                nc.scalar.activation(sqf[:], xT[k][:], AF.Square)
                nc.tensor.matmul(p_mr[:], oc1, sqf[:], start=(k == 0), stop=(k == KT - 1))
            rr = wk.tile([1, T], F32, tag="rr", name="rr")
            nc.scalar.activation(rr[:], p_mr[:], AF.Sqrt, bias=eps1, scale=1.0 / E)
            rr2 = wk.tile([1, T], F32, tag="rr2", name="rr2")
            nc.vector.reciprocal(rr2[:], rr[:])
            rr15 = wk.tile([1, T], F32, tag="rr15", name="rr15")
            nc.vector.tensor_scalar(rr15[:], rr2[:], 1.0 / 15.0, 0.0,
                                    AluOpType.mult, AluOpType.add)
            rcol = []
            for i in range(NTT):
                p_tr = ps1.tile([128, 1], F32, tag="pcs0", name="ptr")
                nc.tensor.transpose(p_tr[:], rr15[:, i * 128:(i + 1) * 128], one_f)
                rc = st.tile([128, 1], F32, tag=f"rcol{i}", name=f"rcol{i}")
                nc.scalar.copy(rc[:], p_tr[:])
                rcol.append(rc)

            for i in range(NTT):
                for v in range(NVT):
                    lmt = lmp.tile([128, KT * 512], F32R, tag="lmt", name="lmt")
                    for k in range(KT):
                        nc.sync.dma_start(
                            lmt[:, k * 512:(k + 1) * 512],
                            d_lm.ap().bitcast(F32R)[k * 128:(k + 1) * 128, v * 512:(v + 1) * 512])
                    p_lg = ps6.tile([128, 512], F32, tag="ps", name="ps")
                    for k in range(KT):
                        nc.tensor.matmul(p_lg[:], xr[k][:, i * 128:(i + 1) * 128],
                                         lmt[:, k * 512:(k + 1) * 512],
                                         start=(k == 0), stop=(k == KT - 1))
                    lth = wk.tile([128, 512], F32, tag="lth", name="lth")
                    nc.scalar.activation(lth[:], p_lg[:], AF.Tanh, scale=rcol[i][:])
                    lt15 = wk.tile([128, 512], F32, tag="lt15", name="lt15")
                    nc.scalar.activation(lt15[:], lth[:], AF.Copy, scale=15.0)
                    nc.sync.dma_start(
                        d_out.ap()[i * 128:(i + 1) * 128, v * 512:(v + 1) * 512],
                        lt15[:])

    nc.compile()
    return nc


def _host_prep(idx, n_steps, wte, adapters, qkv_w, attn_proj, mlp_fc, mlp_proj,
               dep, router_w, router_b, lm_head_w):
    idx = np.asarray(idx)
    wte = np.asarray(wte, np.float32)
    adapters = np.asarray(adapters, np.float32)
    qkv_w = np.asarray(qkv_w, np.float32)
    attn_proj = np.asarray(attn_proj, np.float32)
    mlp_fc = np.asarray(mlp_fc, np.float32)
    mlp_proj = np.asarray(mlp_proj, np.float32)
    dep = np.asarray(dep, np.float32)
    router_w = np.asarray(router_w, np.float32).reshape(E, 1)
    router_b = np.asarray(router_b, np.float32).reshape(-1)
    lm_head_w = np.asarray(lm_head_w, np.float32)
    ns = int(n_steps)

    dp = np.maximum(dep, 0.0)
    depths = np.zeros((N,), np.float32)
    for _ in range(L):
        depths = (dp @ (depths + 1.0)).astype(np.float32)

    w_eff = np.zeros((ns, N), np.float32)
    active_sets = []
    for t in range(ns):
        td = t * (L / ns)
        w_all = np.exp(-np.abs(depths - np.float32(td))).astype(np.float32)
        w = np.where(w_all > 0.15, w_all, 0.0).astype(np.float32)
        w_eff[t] = w
        active_sets.append(tuple(sorted({n // G for n in range(N) if w[n] > 0})))
    active_sets = tuple(active_sets)
    n_ls = max(sum(len(a) for a in active_sets), 1)

    adw = np.zeros((L, 128, KT * 512), np.float32)
    qkw = np.zeros((L, 128, 512), np.float32)
    vw = np.zeros((L, 128, 512), np.float32)
    fcw = np.zeros((L, 128, 1024), np.float32)
    for l in range(L):
        for p in range(PAIRS):
            for o in range(2):
                n = l * G + 2 * p + o
                rows = slice(o * 64, (o + 1) * 64)
                for k in range(KT):
                    adw[l, :, k * 512 + p * 128 + o * 64: k * 512 + p * 128 + (o + 1) * 64] = \
                        adapters[n, :, k * 128:(k + 1) * 128].T
                qkw[l, rows, p * 128:(p + 1) * 128] = qkv_w[n, 0:128, :].T
                vw[l, rows, p * 128 + o * 64: p * 128 + (o + 1) * 64] = qkv_w[n, 128:192, :].T
                fcw[l, rows, p * 256:(p + 1) * 256] = mlp_fc[n].T
    adw, qkw, vw, fcw = _trunc(adw), _trunc(qkw), _trunc(vw), _trunc(fcw)

    cstr = np.zeros((128, 640), np.float32)
    permM = np.zeros((128, 128), np.float32)
    for m in range(128):
        kk = (m // 64) * 64 + ((m % 64) + HD) % 64
        permM[kk, m] = 1.0
    cstr[:, 0:128] = permM
    ob = np.zeros((128, 128), np.float32)
    ob[0:64, 0:64] = 1.0 / GD
    ob[64:128, 64:128] = 1.0 / GD
    cstr[:, 128:256] = ob
    cstr[:, 256:384] = 1.0
    cstr[:, 384:385] = 1.0
    cstr[0, 385:449] = 1.0
    cstr[1, 449:513] = 1.0
    cstr[0:64, 513] = 1.0 / GD
    cstr[64:128, 514] = 1.0 / GD
    cstr = _trunc(cstr)

    inv_freq = 1.0 / (10000.0 ** (np.arange(0, GD, 2, dtype=np.float64) / GD))
    freqs = np.outer(np.arange(T), inv_freq)
    cosT = np.cos(freqs).astype(np.float32).T
    sinT = np.sin(freqs).astype(np.float32).T
    cstf = np.zeros((128, 648), np.float32)
    cstf[:, 640] = EPS
    for blk in range(4):
        cstf[blk * 32:(blk + 1) * 32, 0:256] = cosT
        cstf[blk * 32:(blk + 1) * 32, 256:512] = sinT * (1.0 if blk % 2 == 0 else -1.0)
    s_i = np.arange(128)[:, None]
    t_i = np.arange(128)[None, :]
    cstf[:, 512:640] = (s_i <= t_i).astype(np.float32)

    w_ap = attn_proj.sum(axis=2)
    w_mp = mlp_proj.sum(axis=2)
    wapP = np.zeros((128, L * PAIRS), np.float32)
    wawP = np.zeros((128, n_ls * PAIRS), np.float32)
    wmwP = np.zeros((128, n_ls * PAIRS), np.float32)
    for l in range(L):
        for p in range(PAIRS):
            for o in range(2):
                n = l * G + 2 * p + o
                wapP[o * 64:(o + 1) * 64, l * PAIRS + p] = w_ap[n]
    ls = 0
    for t, layers in enumerate(active_sets):
        for l in layers:
            for p in range(PAIRS):
                for o in range(2):
                    n = l * G + 2 * p + o
                    wawP[o * 64:(o + 1) * 64, ls * PAIRS + p] = w_ap[n] * w_eff[t, n]
                    wmwP[o * 64:(o + 1) * 64, ls * PAIRS + p] = w_mp[n] * w_eff[t, n]
            ls += 1

    rwP = np.zeros((128, KT), np.float32)
    for k in range(KT):
        rwP[:, k] = router_w[k * 128:(k + 1) * 128, 0]
    rwP = _trunc(rwP)
    rbias = np.full((1, 1), np.float32(router_b[0]), np.float32)

    gathered = wte[idx]
    in_maps = []
    for c in range(NC):
        b, vs = c // VSH, c % VSH
        lo = vs * VW
        hi = min(lo + VW, V)
        lmt = np.zeros((E, VQ), np.float32)
        lmt[:, 0:hi - lo] = lm_head_w[lo:hi, :].T
        in_maps.append({
            "x0t": np.ascontiguousarray(gathered[b].T), "adw": adw, "qkw": qkw,
            "vw": vw, "fcw": fcw, "cstr": cstr, "cstf": cstf, "wapP": wapP,
            "wawP": wawP, "wmwP": wmwP, "rwP": rwP, "rbias": rbias,
            "lmt": _trunc(lmt),
        })
    return active_sets, in_maps


def kernel(idx, n_steps, wte, adapters, qkv_w, attn_proj, mlp_fc, mlp_proj,
           dep, router_w, router_b, lm_head_w):
    active_sets, in_maps = _host_prep(
        idx, n_steps, wte, adapters, qkv_w, attn_proj, mlp_fc, mlp_proj,
        dep, router_w, router_b, lm_head_w)

    if active_sets not in _PROGRAM_CACHE:
        _PROGRAM_CACHE[active_sets] = _build_program(active_sets)
    nc = _PROGRAM_CACHE[active_sets]

    trace = bool(int(os.environ.get("BASS_KERNEL_TRACE", "0")))
    res = run_bass_kernel_spmd(nc, in_maps, list(range(NC)), trace=trace)
    if trace and res.exec_time_ns is not None:
        print(f"HW exec time: {res.exec_time_ns} ns")

    out = np.zeros((B, T, V), np.float32)
    for c in range(NC):
        b, vs = c // VSH, c % VSH
        lo = vs * VW
        hi = min(lo + VW, V)
        out[b, :, lo:hi] = res.results[c]["out_lg"][:, 0:hi - lo]
    return out

